# revision 21
# baseline (speedup 1.0000x reference)
"""Bass/Trainium2 kernel for nn_CausalWeighting.

Reference computation (per batch b, edge e):
    s = sigmoid(relu([f[src], f[dst]] @ W1 + b1) @ W2 + b2)
    out[b, src[e], dst[e]] = s[b, e]   (zeros elsewhere)

Restructuring:
  * W1 splits into top/bottom halves so the edge MLP becomes
    h[e] = relu(A[src[e]] + C[dst[e]] + b1), s[e] = sigmoid(h[e] @ W2 + b2)
    with small dense per-node tables A = f @ W1[:D] (+b1), C = f @ W1[D:]
    computed on the tensor engine in feature-major [d, node] layout.
  * Edges are deduplicated per (src, dst) cell (duplicates produce identical
    s) and bucketed by src % 128 into a slot grid [128 partitions, G slots]
    so that every edge's s value is computed on the partition its output row
    lives on. Unused slots hold dummy edges.
  * Per 512-slot tile: gpsimd ap_gather pulls A[src]/C[dst] columns from the
    SBUF-resident tables, DVE adds, ACT relus, and the W2 dot-product runs as
    K-contraction matmuls (lhsT=h block, rhs=W2) writing s columns to PSUM.
  * One sigmoid over the whole [128, G] s grid, then 4 gpsimd local_scatter
    ops (per-partition scatter of the fp32 bit-halves as int16 pairs; the
    instruction zero-fills its destination) build the dense [512, 512] output
    in SBUF, which 4 plain DMAs write out. Exact fp32 end to end.

Sharding: data-parallel over batch, one batch per core (8 cores).
"""

import sys

import numpy as np

if "/opt/trn_rl_repo" not in sys.path:
    sys.path.insert(0, "/opt/trn_rl_repo")

import concourse.bacc as bacc
import concourse.bass as bass
import concourse.mybir as mybir
import concourse.tile as tile
from concourse.bass_utils import run_bass_kernel_spmd

B, N, D, E = 8, 512, 256, 16384
NCORES = 8
T = 1024             # slots per tile
FP = mybir.dt.float32
I16 = mybir.dt.int16
AF = mybir.ActivationFunctionType

_cache = {}


DMA_FRAC = 0.5        # fraction of slots gathered via DMA instead of gpsimd
TD = 1024             # slots per DMA-gather tile


def build_program(G, nd):
    """G: slots/partition (mult of 8). nd leading TD-slot tiles use the DMA
    gather path (dma_gather from DRAM node-major tables + PE transpose);
    the rest use gpsimd ap_gather from SBUF feature-major tables."""
    assert G % 8 == 0
    EG = 128 * G          # total slots
    assert EG % T == 0 and TD % T == 0 and nd * TD <= EG
    nc = bacc.Bacc(None)

    f_ext = nc.declare_dram_parameter("f", [N, D], FP, isOutput=False)
    w1_ext = nc.declare_dram_parameter("w1", [128, 4, D], FP, isOutput=False)
    b1_ext = nc.declare_dram_parameter("b1t", [128, 2], FP, isOutput=False)
    w2_ext = nc.declare_dram_parameter("w2t", [128, 2], FP, isOutput=False)
    b2_ext = nc.declare_dram_parameter("b2t", [128, 1], FP, isOutput=False)
    id_ext = nc.declare_dram_parameter("ident", [128, 128], FP, isOutput=False)
    srcw_ext = nc.declare_dram_parameter("srcw", [128, EG // 16], I16, isOutput=False)
    dstw_ext = nc.declare_dram_parameter("dstw", [128, EG // 16], I16, isOutput=False)
    ls_ext = nc.declare_dram_parameter("lsidx", [128, 4, 2 * G], I16, isOutput=False)
    b1r_ext = nc.declare_dram_parameter("b1rep", [128, D], FP, isOutput=False)
    out_ext = nc.declare_dram_parameter("out", [N, N], FP, isOutput=True)
    a_nm = nc.dram_tensor("a_nm", [N, D], FP)
    c_nm = nc.dram_tensor("c_nm", [N, D], FP)

    with tile.TileContext(nc) as tc:
        with (
            tc.tile_pool(name="const", bufs=1) as cpool,
            tc.tile_pool(name="work", bufs=2) as wpool,
            tc.tile_pool(name="dwork", bufs=3) as dpool,
            tc.tile_pool(name="ps", bufs=4, space="PSUM") as pspool,
            tc.tile_pool(name="psx", bufs=2, space="PSUM") as psxpool,
            tc.tile_pool(name="pss", bufs=1, space="PSUM") as psspool,
        ):
            # ---- load constants / inputs ----
            # f/w1 ride the gpsimd SWDGE queue in parallel with the HWDGE
            # loads; scatter index tables (needed last) load last.
            id_sb = cpool.tile([128, 128], FP)
            nc.sync.dma_start(out=id_sb[:], in_=id_ext[:])
            f_sb = cpool.tile([128, 4, D], FP)        # f[c*128+p, d]
            nc.sync.dma_start(out=f_sb[:], in_=f_ext[:].rearrange("(c p) d -> p c d", p=128))
            w1_sb = cpool.tile([128, 4, D], FP)       # W1[k*128+p, d]
            nc.sync.dma_start(out=w1_sb[:], in_=w1_ext[:])
            b1r_sb = cpool.tile([128, D], FP)
            nc.sync.dma_start(out=b1r_sb[:], in_=b1r_ext[:])
            b1_sb = cpool.tile([128, 2], FP)
            nc.sync.dma_start(out=b1_sb[:], in_=b1_ext[:])
            w2_sb = cpool.tile([128, 2], FP)
            nc.sync.dma_start(out=w2_sb[:], in_=w2_ext[:])
            b2_sb = cpool.tile([128, 1], FP)
            nc.sync.dma_start(out=b2_sb[:], in_=b2_ext[:])
            srcw_sb = cpool.tile([128, EG // 16], I16)
            nc.sync.dma_start(out=srcw_sb[:], in_=srcw_ext[:])
            dstw_sb = cpool.tile([128, EG // 16], I16)
            nc.sync.dma_start(out=dstw_sb[:], in_=dstw_ext[:])
            ls_sb = cpool.tile([128, 4, 2 * G], I16)
            nc.sync.dma_start(out=ls_sb[:], in_=ls_ext[:])

            # ---- fT[k, n] = f[n, k] via PE transpose of [128,128] blocks ----
            ft_sb = cpool.tile([128, 2, N], FP)       # fT[k*128+p, n]
            for c in range(4):        # node chunk
                for k in range(2):    # feature chunk
                    ps = pspool.tile([128, 128], FP, tag="tbl")
                    nc.tensor.transpose(ps[:], f_sb[:, c, k * 128:(k + 1) * 128], id_sb[:])
                    nc.scalar.copy(ft_sb[:, k, c * 128:(c + 1) * 128], ps[:])

            # ---- node-major tables A_nm (+b1), C_nm -> DRAM ----
            if nd > 0:
                for c in range(4):    # node chunk
                    psn = pspool.tile([128, D], FP, tag="tbl")
                    for k in range(2):
                        nc.tensor.matmul(psn[:], lhsT=ft_sb[:, k, c * 128:(c + 1) * 128],
                                         rhs=w1_sb[:, k, :], start=(k == 0), stop=(k == 1))
                    sta = wpool.tile([128, D], FP, tag="tstage")
                    nc.vector.tensor_add(sta[:], psn[:], b1r_sb[:])
                    nc.sync.dma_start(out=a_nm[c * 128:(c + 1) * 128, :], in_=sta[:])
                    psn2 = pspool.tile([128, D], FP, tag="tbl")
                    for k in range(2):
                        nc.tensor.matmul(psn2[:], lhsT=ft_sb[:, k, c * 128:(c + 1) * 128],
                                         rhs=w1_sb[:, 2 + k, :], start=(k == 0), stop=(k == 1))
                    stc = wpool.tile([128, D], FP, tag="tstage")
                    nc.scalar.copy(stc[:], psn2[:])
                    nc.sync.dma_start(out=c_nm[c * 128:(c + 1) * 128, :], in_=stc[:])

            # ---- tables A^T (bias folded) and C^T, layout [d, n] ----
            at_sb = cpool.tile([128, 2, N], FP)
            ct_sb = cpool.tile([128, 2, N], FP)
            for m in range(2):        # d chunk
                psa = pspool.tile([128, N], FP, tag="tbl")
                for k in range(2):
                    nc.tensor.matmul(psa[:], lhsT=w1_sb[:, k, m * 128:(m + 1) * 128],
                                     rhs=ft_sb[:, k, :], start=(k == 0), stop=(k == 1))
                nc.scalar.activation(at_sb[:, m, :], psa[:], AF.Identity,
                                     bias=b1_sb[:, m:m + 1])
                psc = pspool.tile([128, N], FP, tag="tbl")
                for k in range(2):
                    nc.tensor.matmul(psc[:], lhsT=w1_sb[:, 2 + k, m * 128:(m + 1) * 128],
                                     rhs=ft_sb[:, k, :], start=(k == 0), stop=(k == 1))
                nc.scalar.copy(ct_sb[:, m, :], psc[:])

            # ---- per-slot phase: s logits into [128, G] psum grid ----
            ndc = nd * TD // 128                  # s columns covered by DMA tiles
            s_psd = psspool.tile([128, max(ndc, 2)], FP, tag="spsd")
            s_psp = psspool.tile([128, max(G - ndc, 2)], FP, tag="spsp")

            def dma_tile(base):
                ic0 = base // 16
                gA = dpool.tile([128, TD // 128, D], FP, tag="gA")
                gC = dpool.tile([128, TD // 128, D], FP, tag="gC")
                nc.gpsimd.dma_gather(gA[:], a_nm[:], srcw_sb[:, ic0:ic0 + TD // 16],
                                     num_idxs=TD, num_idxs_reg=TD, elem_size=D,
                                     elem_step=D)
                nc.gpsimd.dma_gather(gC[:], c_nm[:], dstw_sb[:, ic0:ic0 + TD // 16],
                                     num_idxs=TD, num_idxs_reg=TD, elem_size=D,
                                     elem_step=D)
                xe = gA
                nc.vector.tensor_add(xe[:], gA[:], gC[:])
                for sub in range(TD // 512):          # 512-slot subgroups
                    hts = []
                    for k in range(2):                # feature chunk
                        psb = psxpool.tile([128, 512], FP, tag="xb")
                        for b in range(4):            # 128-slot block
                            nc.tensor.transpose(
                                psb[:, b * 128:(b + 1) * 128],
                                xe[:, sub * 4 + b, k * 128:(k + 1) * 128], id_sb[:])
                        ht = dpool.tile([128, 512], FP, tag=f"hd{k}")
                        nc.scalar.activation(ht[:], psb[:], AF.Relu)
                        hts.append(ht)
                    for blk in range(4):
                        g = (base + sub * 512 + blk * 128) // 128
                        sl = slice(blk * 128, (blk + 1) * 128)
                        nc.tensor.matmul(s_psd[:, g:g + 1], lhsT=hts[0][:, sl],
                                         rhs=w2_sb[:, 0:1], start=True, stop=False)
                        nc.tensor.matmul(s_psd[:, g:g + 1], lhsT=hts[1][:, sl],
                                         rhs=w2_sb[:, 1:2], start=False, stop=True)

            def pool_tile(base):
                ic0 = base // 16
                isl = slice(ic0, ic0 + T // 16)
                ga0 = wpool.tile([128, T], FP, tag="ga0")
                ga1 = wpool.tile([128, T], FP, tag="ga1")
                gc0 = wpool.tile([128, T], FP, tag="gc0")
                gc1 = wpool.tile([128, T], FP, tag="gc1")
                nc.gpsimd.ap_gather(ga0[:], at_sb[:, 0, :], srcw_sb[:, isl],
                                    channels=128, num_elems=N, d=1, num_idxs=T)
                nc.gpsimd.ap_gather(ga1[:], at_sb[:, 1, :], srcw_sb[:, isl],
                                    channels=128, num_elems=N, d=1, num_idxs=T)
                nc.gpsimd.ap_gather(gc0[:], ct_sb[:, 0, :], dstw_sb[:, isl],
                                    channels=128, num_elems=N, d=1, num_idxs=T)
                nc.gpsimd.ap_gather(gc1[:], ct_sb[:, 1, :], dstw_sb[:, isl],
                                    channels=128, num_elems=N, d=1, num_idxs=T)
                x0 = wpool.tile([128, T], FP, tag="x0")
                x1 = wpool.tile([128, T], FP, tag="x1")
                nc.vector.tensor_add(x0[:], ga0[:], gc0[:])
                nc.vector.tensor_add(x1[:], ga1[:], gc1[:])
                h0 = wpool.tile([128, T], FP, tag="h0")
                h1 = wpool.tile([128, T], FP, tag="h1")
                nc.scalar.activation(h0[:], x0[:], AF.Relu)
                nc.scalar.activation(h1[:], x1[:], AF.Relu)
                for blk in range(T // 128):
                    g = base // 128 + blk - ndc
                    sl = slice(blk * 128, (blk + 1) * 128)
                    nc.tensor.matmul(s_psp[:, g:g + 1], lhsT=h0[:, sl], rhs=w2_sb[:, 0:1],
                                     start=True, stop=False)
                    nc.tensor.matmul(s_psp[:, g:g + 1], lhsT=h1[:, sl], rhs=w2_sb[:, 1:2],
                                     start=False, stop=True)

            # interleave: DMA tiles cover [0, nd*TD), pool tiles the rest
            np_tiles = (EG - nd * TD) // T
            order = []
            pi, di = 0, 0
            while di < min(3, nd):
                order.append(("d", di * TD))
                di += 1
            while di < nd or pi < np_tiles:
                if di < nd:
                    order.append(("d", di * TD))
                    di += 1
                if pi < np_tiles:
                    order.append(("p", nd * TD + pi * T))
                    pi += 1
            for kind, base in order:
                if kind == "d":
                    dma_tile(base)
                else:
                    pool_tile(base)

            s_sb = cpool.tile([128, G], FP)
            if ndc > 0:
                nc.scalar.activation(s_sb[:, :ndc], s_psd[:, :ndc], AF.Sigmoid,
                                     bias=b2_sb[:, 0:1])
            if G - ndc > 0:
                nc.scalar.activation(s_sb[:, ndc:], s_psp[:, :G - ndc], AF.Sigmoid,
                                     bias=b2_sb[:, 0:1])
            s16 = s_sb[:].bitcast(I16)                # [128, 2G] bit halves

            # ---- build dense output rows in SBUF and write out ----
            for c in range(4):
                dense = wpool.tile([128, 2 * N], I16, tag=f"dense{c}")
                nc.gpsimd.local_scatter(dense[:], s16, ls_sb[:, c, :],
                                        channels=128, num_elems=2 * N, num_idxs=2 * G)
                nc.sync.dma_start(out=out_ext[c * 128:(c + 1) * 128, :],
                                  in_=dense[:].bitcast(FP))

    nc.compile()
    return nc


def _prep_host(features, W1, b1, W2, b2, edge_index):
    f = np.ascontiguousarray(np.asarray(features, dtype=np.float32))
    W1 = np.asarray(W1, dtype=np.float32)
    b1 = np.asarray(b1, dtype=np.float32)
    W2 = np.asarray(W2, dtype=np.float32)
    b2 = np.asarray(b2, dtype=np.float32)
    ei = np.asarray(edge_index).astype(np.int64)
    src, dst = ei[0], ei[1]

    w1r = np.ascontiguousarray(W1.reshape(4, 128, D).transpose(1, 0, 2))
    b1t = np.ascontiguousarray(b1.reshape(2, 128).T)
    w2t = np.ascontiguousarray(W2.reshape(2, 128, 1)[:, :, 0].T)
    b2t = np.full((128, 1), b2.reshape(-1)[0], dtype=np.float32)
    ident = np.eye(128, dtype=np.float32)

    # dedup: keep last occurrence per (src, dst) cell
    flat = src * N + dst
    keep = np.zeros(E, dtype=bool)
    _, first_of_rev = np.unique(flat[::-1], return_index=True)
    keep[E - 1 - first_of_rev] = True
    ks, kd = src[keep], dst[keep]

    # bucket kept edges by src % 128 into slot grid [128, G]
    part = (ks % 128).astype(np.int64)
    order = np.argsort(part, kind="stable")
    part_s, ks_s, kd_s = part[order], ks[order], kd[order]
    counts = np.bincount(part_s, minlength=128)
    Gmax = int(counts.max())
    G = max(8, (Gmax + 7) // 8 * 8)

    sl_src = np.zeros((128, G), np.int64)
    sl_dst = np.zeros((128, G), np.int64)
    filled = np.zeros((128, G), bool)
    starts = np.zeros(129, np.int64)
    np.cumsum(counts, out=starts[1:])
    slot_m = np.arange(len(ks_s)) - starts[part_s]      # position within bucket
    sl_src[part_s, slot_m] = ks_s
    sl_dst[part_s, slot_m] = kd_s
    filled[part_s, slot_m] = True

    EG = 128 * G

    def wrap(grid):
        # slot-edge e'' = m*128 + p -> global 16-wrap [128, EG//16] int16:
        # column c, partition 16g+pp  ->  e'' = c*16 + pp  (same for all g)
        a = np.ascontiguousarray(grid.T.reshape(-1)).astype(np.int16)   # flat[e'']
        a16 = a.reshape(EG // 16, 16).T
        return np.ascontiguousarray(np.tile(a16, (8, 1)))

    srcw, dstw = wrap(sl_src), wrap(sl_dst)

    # local_scatter index tables: chunk c gets edges with src//128 == c
    chunk = sl_src >> 7
    lsidx = np.full((128, 4, 2 * G), -1, np.int16)
    pp, mm = np.nonzero(filled)
    cc = chunk[pp, mm]
    lsidx[pp, cc, 2 * mm] = (2 * sl_dst[pp, mm]).astype(np.int16)
    lsidx[pp, cc, 2 * mm + 1] = (2 * sl_dst[pp, mm] + 1).astype(np.int16)

    b1rep = np.ascontiguousarray(np.broadcast_to(b1[None, :], (128, D)))
    shared = {"w1": w1r, "b1t": b1t, "w2t": w2t, "b2t": b2t, "ident": ident,
              "srcw": srcw, "dstw": dstw, "lsidx": lsidx, "b1rep": b1rep}
    in_maps = [dict(shared, f=np.ascontiguousarray(f[b])) for b in range(B)]
    return G, in_maps


def n_dma_tiles(G):
    EG = 128 * G
    return int(round(DMA_FRAC * EG / TD))


def kernel(features, W1, b1, W2, b2, edge_index):
    G, in_maps = _prep_host(features, W1, b1, W2, b2, edge_index)
    key = (G, n_dma_tiles(G))
    if key not in _cache:
        _cache[key] = build_program(G, n_dma_tiles(G))
    nc = _cache[key]
    res = run_bass_kernel_spmd(nc, in_maps, list(range(NCORES)))
    out = np.stack([res.results[c]["out"] for c in range(NCORES)], axis=0)
    return out.astype(np.float32)


# revision 27
# speedup vs baseline: 1.0148x; 1.0148x over previous
"""Bass/Trainium2 kernel for nn_CausalWeighting.

Reference computation (per batch b, edge e):
    s = sigmoid(relu([f[src], f[dst]] @ W1 + b1) @ W2 + b2)
    out[b, src[e], dst[e]] = s[b, e]   (zeros elsewhere)

Restructuring:
  * W1 splits into top/bottom halves so the edge MLP becomes
    h[e] = relu(A[src[e]] + C[dst[e]] + b1), s[e] = sigmoid(h[e] @ W2 + b2)
    with small dense per-node tables A = f @ W1[:D] (+b1), C = f @ W1[D:]
    computed on the tensor engine in feature-major [d, node] layout.
  * Edges are deduplicated per (src, dst) cell (duplicates produce identical
    s) and bucketed by src % 128 into a slot grid [128 partitions, G slots]
    so that every edge's s value is computed on the partition its output row
    lives on. Unused slots hold dummy edges.
  * Per 512-slot tile: gpsimd ap_gather pulls A[src]/C[dst] columns from the
    SBUF-resident tables, DVE adds, ACT relus, and the W2 dot-product runs as
    K-contraction matmuls (lhsT=h block, rhs=W2) writing s columns to PSUM.
  * One sigmoid over the whole [128, G] s grid, then 4 gpsimd local_scatter
    ops (per-partition scatter of the fp32 bit-halves as int16 pairs; the
    instruction zero-fills its destination) build the dense [512, 512] output
    in SBUF, which 4 plain DMAs write out. Exact fp32 end to end.

Sharding: data-parallel over batch, one batch per core (8 cores).
"""

import sys

import numpy as np

if "/opt/trn_rl_repo" not in sys.path:
    sys.path.insert(0, "/opt/trn_rl_repo")

import concourse.bacc as bacc
import concourse.bass as bass
import concourse.mybir as mybir
import concourse.tile as tile
from concourse.bass_utils import run_bass_kernel_spmd

B, N, D, E = 8, 512, 256, 16384
NCORES = 8
T = 1024             # slots per tile
FP = mybir.dt.float32
I16 = mybir.dt.int16
AF = mybir.ActivationFunctionType

_cache = {}


DMA_FRAC = 0.5        # fraction of slots gathered via DMA instead of gpsimd
TD = 1024             # slots per DMA-gather tile


def build_program(G, nd):
    """G: slots/partition (mult of 8). nd leading TD-slot tiles use the DMA
    gather path (dma_gather from DRAM node-major tables + PE transpose);
    the rest use gpsimd ap_gather from SBUF feature-major tables."""
    assert G % 8 == 0
    EG = 128 * G          # total slots
    assert EG % T == 0 and TD % T == 0 and nd * TD <= EG
    nc = bacc.Bacc(None)

    f_ext = nc.declare_dram_parameter("f", [N, D], FP, isOutput=False)
    w1_ext = nc.declare_dram_parameter("w1", [128, 4, D], FP, isOutput=False)
    b1_ext = nc.declare_dram_parameter("b1t", [128, 2], FP, isOutput=False)
    w2_ext = nc.declare_dram_parameter("w2t", [128, 2], FP, isOutput=False)
    b2_ext = nc.declare_dram_parameter("b2t", [128, 1], FP, isOutput=False)
    id_ext = nc.declare_dram_parameter("ident", [128, 128], FP, isOutput=False)
    srcw_ext = nc.declare_dram_parameter("srcw", [128, EG // 16], I16, isOutput=False)
    dstw_ext = nc.declare_dram_parameter("dstw", [128, EG // 16], I16, isOutput=False)
    ls_ext = nc.declare_dram_parameter("lsidx", [128, 4, 2 * G], I16, isOutput=False)
    b1r_ext = nc.declare_dram_parameter("b1rep", [128, D], FP, isOutput=False)
    out_ext = nc.declare_dram_parameter("out", [N, N], FP, isOutput=True)
    a_nm = nc.dram_tensor("a_nm", [N, D], FP)
    c_nm = nc.dram_tensor("c_nm", [N, D], FP)

    with tile.TileContext(nc) as tc:
        with (
            tc.tile_pool(name="const", bufs=1) as cpool,
            tc.tile_pool(name="work", bufs=2) as wpool,
            tc.tile_pool(name="dwork", bufs=3) as dpool,
            tc.tile_pool(name="ps", bufs=4, space="PSUM") as pspool,
            tc.tile_pool(name="psx", bufs=2, space="PSUM") as psxpool,
            tc.tile_pool(name="pss", bufs=1, space="PSUM") as psspool,
        ):
            # ---- load constants / inputs ----
            # f/w1 ride the gpsimd SWDGE queue in parallel with the HWDGE
            # loads; scatter index tables (needed last) load last.
            id_sb = cpool.tile([128, 128], FP)
            nc.sync.dma_start(out=id_sb[:], in_=id_ext[:])
            f_sb = cpool.tile([128, 4, D], FP)        # f[c*128+p, d]
            nc.sync.dma_start(out=f_sb[:], in_=f_ext[:].rearrange("(c p) d -> p c d", p=128))
            w1_sb = cpool.tile([128, 4, D], FP)       # W1[k*128+p, d]
            nc.sync.dma_start(out=w1_sb[:], in_=w1_ext[:])
            b1r_sb = cpool.tile([128, D], FP)
            nc.sync.dma_start(out=b1r_sb[:], in_=b1r_ext[:])
            b1_sb = cpool.tile([128, 2], FP)
            nc.sync.dma_start(out=b1_sb[:], in_=b1_ext[:])
            w2_sb = cpool.tile([128, 2], FP)
            nc.sync.dma_start(out=w2_sb[:], in_=w2_ext[:])
            b2_sb = cpool.tile([128, 1], FP)
            nc.sync.dma_start(out=b2_sb[:], in_=b2_ext[:])
            srcw_sb = cpool.tile([128, EG // 16], I16)
            nc.sync.dma_start(out=srcw_sb[:], in_=srcw_ext[:])
            dstw_sb = cpool.tile([128, EG // 16], I16)
            nc.sync.dma_start(out=dstw_sb[:], in_=dstw_ext[:])
            ls_sb = cpool.tile([128, 4, 2 * G], I16)
            nc.sync.dma_start(out=ls_sb[:], in_=ls_ext[:])

            # ---- fT[k, n] = f[n, k] via PE transpose of [128,128] blocks ----
            ft_sb = cpool.tile([128, 2, N], FP)       # fT[k*128+p, n]
            for c in range(4):        # node chunk
                for k in range(2):    # feature chunk
                    ps = pspool.tile([128, 128], FP, tag="tbl")
                    nc.tensor.transpose(ps[:], f_sb[:, c, k * 128:(k + 1) * 128], id_sb[:])
                    nc.scalar.copy(ft_sb[:, k, c * 128:(c + 1) * 128], ps[:])

            # ---- node-major tables A_nm (+b1), C_nm -> DRAM ----
            if nd > 0:
                sta = cpool.tile([128, 4, D], FP)
                stc = cpool.tile([128, 4, D], FP)
                for c in range(4):    # node chunk
                    psn = pspool.tile([128, D], FP, tag="tbl")
                    for k in range(2):
                        nc.tensor.matmul(psn[:], lhsT=ft_sb[:, k, c * 128:(c + 1) * 128],
                                         rhs=w1_sb[:, k, :], start=(k == 0), stop=(k == 1))
                    nc.vector.tensor_add(sta[:, c, :], psn[:], b1r_sb[:])
                    psn2 = pspool.tile([128, D], FP, tag="tbl")
                    for k in range(2):
                        nc.tensor.matmul(psn2[:], lhsT=ft_sb[:, k, c * 128:(c + 1) * 128],
                                         rhs=w1_sb[:, 2 + k, :], start=(k == 0), stop=(k == 1))
                    nc.scalar.copy(stc[:, c, :], psn2[:])
                nc.sync.dma_start(out=a_nm[:].rearrange("(c p) d -> p c d", p=128), in_=sta[:])
                nc.sync.dma_start(out=c_nm[:].rearrange("(c p) d -> p c d", p=128), in_=stc[:])

            # ---- tables A^T (bias folded) and C^T, layout [d, n] ----
            at_sb = cpool.tile([128, 2, N], FP)
            ct_sb = cpool.tile([128, 2, N], FP)
            for m in range(2):        # d chunk
                psa = pspool.tile([128, N], FP, tag="tbl")
                for k in range(2):
                    nc.tensor.matmul(psa[:], lhsT=w1_sb[:, k, m * 128:(m + 1) * 128],
                                     rhs=ft_sb[:, k, :], start=(k == 0), stop=(k == 1))
                nc.scalar.activation(at_sb[:, m, :], psa[:], AF.Identity,
                                     bias=b1_sb[:, m:m + 1])
                psc = pspool.tile([128, N], FP, tag="tbl")
                for k in range(2):
                    nc.tensor.matmul(psc[:], lhsT=w1_sb[:, 2 + k, m * 128:(m + 1) * 128],
                                     rhs=ft_sb[:, k, :], start=(k == 0), stop=(k == 1))
                nc.scalar.copy(ct_sb[:, m, :], psc[:])

            # ---- per-slot phase: s logits into [128, G] psum grid ----
            ndc = nd * TD // 128                  # s columns covered by DMA tiles
            s_psd = psspool.tile([128, max(ndc, 2)], FP, tag="spsd")
            s_psp = psspool.tile([128, max(G - ndc, 2)], FP, tag="spsp")

            def dma_tile(base):
                ic0 = base // 16
                gA = dpool.tile([128, TD // 128, D], FP, tag="gA")
                gC = dpool.tile([128, TD // 128, D], FP, tag="gC")
                nc.gpsimd.dma_gather(gA[:], a_nm[:], srcw_sb[:, ic0:ic0 + TD // 16],
                                     num_idxs=TD, num_idxs_reg=TD, elem_size=D,
                                     elem_step=D)
                nc.gpsimd.dma_gather(gC[:], c_nm[:], dstw_sb[:, ic0:ic0 + TD // 16],
                                     num_idxs=TD, num_idxs_reg=TD, elem_size=D,
                                     elem_step=D)
                xe = gA
                nc.vector.tensor_add(xe[:], gA[:], gC[:])
                for sub in range(TD // 512):          # 512-slot subgroups
                    hts = []
                    for k in range(2):                # feature chunk
                        psb = psxpool.tile([128, 512], FP, tag="xb")
                        for b in range(4):            # 128-slot block
                            nc.tensor.transpose(
                                psb[:, b * 128:(b + 1) * 128],
                                xe[:, sub * 4 + b, k * 128:(k + 1) * 128], id_sb[:])
                        ht = dpool.tile([128, 512], FP, tag=f"hd{k}")
                        nc.scalar.activation(ht[:], psb[:], AF.Relu)
                        hts.append(ht)
                    for blk in range(4):
                        g = (base + sub * 512 + blk * 128) // 128
                        sl = slice(blk * 128, (blk + 1) * 128)
                        nc.tensor.matmul(s_psd[:, g:g + 1], lhsT=hts[0][:, sl],
                                         rhs=w2_sb[:, 0:1], start=True, stop=False)
                        nc.tensor.matmul(s_psd[:, g:g + 1], lhsT=hts[1][:, sl],
                                         rhs=w2_sb[:, 1:2], start=False, stop=True)

            def pool_tile(base):
                ic0 = base // 16
                isl = slice(ic0, ic0 + T // 16)
                ga0 = wpool.tile([128, T], FP, tag="ga0")
                ga1 = wpool.tile([128, T], FP, tag="ga1")
                gc0 = wpool.tile([128, T], FP, tag="gc0")
                gc1 = wpool.tile([128, T], FP, tag="gc1")
                nc.gpsimd.ap_gather(ga0[:], at_sb[:, 0, :], srcw_sb[:, isl],
                                    channels=128, num_elems=N, d=1, num_idxs=T)
                nc.gpsimd.ap_gather(ga1[:], at_sb[:, 1, :], srcw_sb[:, isl],
                                    channels=128, num_elems=N, d=1, num_idxs=T)
                nc.gpsimd.ap_gather(gc0[:], ct_sb[:, 0, :], dstw_sb[:, isl],
                                    channels=128, num_elems=N, d=1, num_idxs=T)
                nc.gpsimd.ap_gather(gc1[:], ct_sb[:, 1, :], dstw_sb[:, isl],
                                    channels=128, num_elems=N, d=1, num_idxs=T)
                x0 = wpool.tile([128, T], FP, tag="x0")
                x1 = wpool.tile([128, T], FP, tag="x1")
                nc.vector.tensor_add(x0[:], ga0[:], gc0[:])
                nc.vector.tensor_add(x1[:], ga1[:], gc1[:])
                h0 = wpool.tile([128, T], FP, tag="h0")
                h1 = wpool.tile([128, T], FP, tag="h1")
                nc.scalar.activation(h0[:], x0[:], AF.Relu)
                nc.scalar.activation(h1[:], x1[:], AF.Relu)
                for blk in range(T // 128):
                    g = base // 128 + blk - ndc
                    sl = slice(blk * 128, (blk + 1) * 128)
                    nc.tensor.matmul(s_psp[:, g:g + 1], lhsT=h0[:, sl], rhs=w2_sb[:, 0:1],
                                     start=True, stop=False)
                    nc.tensor.matmul(s_psp[:, g:g + 1], lhsT=h1[:, sl], rhs=w2_sb[:, 1:2],
                                     start=False, stop=True)

            # interleave: DMA tiles cover [0, nd*TD), pool tiles the rest
            np_tiles = (EG - nd * TD) // T
            order = []
            pi, di = 0, 0
            while di < min(2, nd):
                order.append(("d", di * TD))
                di += 1
            while di < nd or pi < np_tiles:
                if di < nd:
                    order.append(("d", di * TD))
                    di += 1
                if pi < np_tiles:
                    order.append(("p", nd * TD + pi * T))
                    pi += 1
            for kind, base in order:
                if kind == "d":
                    dma_tile(base)
                else:
                    pool_tile(base)

            s_sb = cpool.tile([128, G], FP)
            if ndc > 0:
                nc.scalar.activation(s_sb[:, :ndc], s_psd[:, :ndc], AF.Sigmoid,
                                     bias=b2_sb[:, 0:1])
            if G - ndc > 0:
                nc.scalar.activation(s_sb[:, ndc:], s_psp[:, :G - ndc], AF.Sigmoid,
                                     bias=b2_sb[:, 0:1])
            s16 = s_sb[:].bitcast(I16)                # [128, 2G] bit halves

            # ---- build dense output rows in SBUF and write out ----
            for c in range(4):
                dense = wpool.tile([128, 2 * N], I16, tag=f"dense{c}")
                nc.gpsimd.local_scatter(dense[:], s16, ls_sb[:, c, :],
                                        channels=128, num_elems=2 * N, num_idxs=2 * G)
                nc.sync.dma_start(out=out_ext[c * 128:(c + 1) * 128, :],
                                  in_=dense[:].bitcast(FP))

    nc.compile()
    return nc


def _prep_host(features, W1, b1, W2, b2, edge_index):
    f = np.ascontiguousarray(np.asarray(features, dtype=np.float32))
    W1 = np.asarray(W1, dtype=np.float32)
    b1 = np.asarray(b1, dtype=np.float32)
    W2 = np.asarray(W2, dtype=np.float32)
    b2 = np.asarray(b2, dtype=np.float32)
    ei = np.asarray(edge_index).astype(np.int64)
    src, dst = ei[0], ei[1]

    w1r = np.ascontiguousarray(W1.reshape(4, 128, D).transpose(1, 0, 2))
    b1t = np.ascontiguousarray(b1.reshape(2, 128).T)
    w2t = np.ascontiguousarray(W2.reshape(2, 128, 1)[:, :, 0].T)
    b2t = np.full((128, 1), b2.reshape(-1)[0], dtype=np.float32)
    ident = np.eye(128, dtype=np.float32)

    # dedup: keep last occurrence per (src, dst) cell
    flat = src * N + dst
    keep = np.zeros(E, dtype=bool)
    _, first_of_rev = np.unique(flat[::-1], return_index=True)
    keep[E - 1 - first_of_rev] = True
    ks, kd = src[keep], dst[keep]

    # bucket kept edges by src % 128 into slot grid [128, G]
    part = (ks % 128).astype(np.int64)
    order = np.argsort(part, kind="stable")
    part_s, ks_s, kd_s = part[order], ks[order], kd[order]
    counts = np.bincount(part_s, minlength=128)
    Gmax = int(counts.max())
    G = max(8, (Gmax + 7) // 8 * 8)

    sl_src = np.zeros((128, G), np.int64)
    sl_dst = np.zeros((128, G), np.int64)
    filled = np.zeros((128, G), bool)
    starts = np.zeros(129, np.int64)
    np.cumsum(counts, out=starts[1:])
    slot_m = np.arange(len(ks_s)) - starts[part_s]      # position within bucket
    sl_src[part_s, slot_m] = ks_s
    sl_dst[part_s, slot_m] = kd_s
    filled[part_s, slot_m] = True

    EG = 128 * G

    def wrap(grid):
        # slot-edge e'' = m*128 + p -> global 16-wrap [128, EG//16] int16:
        # column c, partition 16g+pp  ->  e'' = c*16 + pp  (same for all g)
        a = np.ascontiguousarray(grid.T.reshape(-1)).astype(np.int16)   # flat[e'']
        a16 = a.reshape(EG // 16, 16).T
        return np.ascontiguousarray(np.tile(a16, (8, 1)))

    srcw, dstw = wrap(sl_src), wrap(sl_dst)

    # local_scatter index tables: chunk c gets edges with src//128 == c
    chunk = sl_src >> 7
    lsidx = np.full((128, 4, 2 * G), -1, np.int16)
    pp, mm = np.nonzero(filled)
    cc = chunk[pp, mm]
    lsidx[pp, cc, 2 * mm] = (2 * sl_dst[pp, mm]).astype(np.int16)
    lsidx[pp, cc, 2 * mm + 1] = (2 * sl_dst[pp, mm] + 1).astype(np.int16)

    b1rep = np.ascontiguousarray(np.broadcast_to(b1[None, :], (128, D)))
    shared = {"w1": w1r, "b1t": b1t, "w2t": w2t, "b2t": b2t, "ident": ident,
              "srcw": srcw, "dstw": dstw, "lsidx": lsidx, "b1rep": b1rep}
    in_maps = [dict(shared, f=np.ascontiguousarray(f[b])) for b in range(B)]
    return G, in_maps


def n_dma_tiles(G):
    EG = 128 * G
    return int(round(DMA_FRAC * EG / TD))


def kernel(features, W1, b1, W2, b2, edge_index):
    G, in_maps = _prep_host(features, W1, b1, W2, b2, edge_index)
    key = (G, n_dma_tiles(G))
    if key not in _cache:
        _cache[key] = build_program(G, n_dma_tiles(G))
    nc = _cache[key]
    res = run_bass_kernel_spmd(nc, in_maps, list(range(NCORES)))
    out = np.stack([res.results[c]["out"] for c in range(NCORES)], axis=0)
    return out.astype(np.float32)
